# revision 36
# baseline (speedup 1.0000x reference)
"""Distributed single-head attention for Trainium2 (8 NeuronCores).

Problem: B=4, S=2048, D=1024 fp32 attention:
    q = x@Wq+bq; k = x@Wk+bk; v = x@Wv+bv
    out = softmax(q k^T / sqrt(D) + mask) v

Sharding: core c handles batch c//2, query rows [1024*(c%2), +1024).
The host gives every core the FULL x of its batch (both query halves),
so there are NO collectives at all: keys/values for all 2048 positions
are derived locally. The extra cost (projecting the peer half of V,
+128 matmuls) is far cheaper than the serialized in-pair AllGathers +
global CC barrier it replaces (~75us on the old critical path).

Algebraic trick (kept from the earlier version) — the K projection is
eliminated: q_i.k_j = x_i (Wq Wk^T) x_j^T + x_j.(Wk bq) + alpha_i where
alpha_i is a per-row constant softmax drops. With host-precomputed
M2 = Wq@Wk^T and w2 = Wk@bq, scores (up to row const) = TT @ x^T where
TT = x_q @ M2 + w2. One projection instead of two.

Precision: the scores matmul runs in fp8e4m3 with MatmulPerfMode
.DoubleRow (two 128-deep k-planes per instruction -> 2x PE rate).
Measured end-to-end rel err 1.0e-2 (vs 2.1e-3 all-bf16; gate 2e-2).
fp8 anywhere on the V/attn path pushes past the gate, so V proj, TT
proj and PV stay bf16.

Per-core phases (single PE stream, ~800 matmuls):
  V proj:   vt[k][128,1024] = xt^T Wv + bv  for all 16 key chunks
  TT proj:  tt8[e2] fp8 [128, 2*1024] = (M2^T xt_own + w2), d'-pairs
  qc loop (8 query chunks of 128, software-pipelined):
    scores[q,s] fp8 DoubleRow: lhsT=tt8 pairs, rhs=x8 pairs
    e = exp(SCALE*s (+mask)), row-sums via ScalarE accum_out
    attnT via one 3D xbar DMA-transpose (bf16)
    o = attnT^T V (bf16, fp32 PSUM), evicted with *1/rowsum fused
"""

from contextlib import ExitStack

import numpy as np
import ml_dtypes

import concourse.bass as bass
import concourse.tile as tile
import concourse.mybir as mybir
from concourse import bacc
from concourse.bass_utils import run_bass_kernel_spmd

BF16 = mybir.dt.bfloat16
FP8 = mybir.dt.float8e4
F32 = mybir.dt.float32
AF = mybir.ActivationFunctionType
DR = mybir.MatmulPerfMode.DoubleRow

D = 1024  # model dim (= contraction dim)
S = 2048  # full sequence (keys)
Q = 1024  # queries per core
P = 128  # partitions
ND = D // P  # 8 d-chunks
NS = S // P  # 16 key chunks
NQ = Q // P  # 8 query chunks
NBF = 10  # key chunks kept bf16 in PV; the rest go fp8 DoubleRow
SCALE = 1.0 / float(np.sqrt(np.float32(D)))

_NC_CACHE: dict[tuple[bool, int], bacc.Bacc] = {}


def _build(use_mask: bool) -> bacc.Bacc:
    """One program for all cores. The host permutes keys so each core's
    own query rows are ALWAYS xt columns 0..1023 (own keys first, peer
    keys after); softmax+PV are key-permutation invariant and the mask
    columns are permuted to match, so a single program serves both
    query halves."""
    nc = bacc.Bacc("TRN2", target_bir_lowering=False, debug=False, num_devices=8)

    xt_d = nc.dram_tensor("xt", [D, S], BF16, kind="ExternalInput")
    x8_d = nc.dram_tensor("x8", [D, S], FP8, kind="ExternalInput")
    m2_d = nc.dram_tensor("m2", [D, D], BF16, kind="ExternalInput")
    wv_d = nc.dram_tensor("wv", [D, D], BF16, kind="ExternalInput")
    w2_d = nc.dram_tensor("w22", [P, ND], F32, kind="ExternalInput")
    if use_mask:
        mask_d = nc.dram_tensor("maskp", [Q, S], F32, kind="ExternalInput")
    out_d = nc.dram_tensor("out", [Q, D], F32, kind="ExternalOutput")

    qoff = 0  # own query rows are always the first 1024 xt columns

    with tile.TileContext(nc) as tc, ExitStack() as ctx:
        xt_pool = ctx.enter_context(tc.tile_pool(name="xt", bufs=ND))
        x8_pool = ctx.enter_context(tc.tile_pool(name="x8", bufs=ND // 2))
        m2_pool = ctx.enter_context(tc.tile_pool(name="m2", bufs=ND))
        wv_pool = ctx.enter_context(tc.tile_pool(name="wv", bufs=ND))
        tt_pool = ctx.enter_context(tc.tile_pool(name="tt", bufs=ND // 2))
        vt_pool = ctx.enter_context(tc.tile_pool(name="vt", bufs=NBF))
        v8_pool = ctx.enter_context(tc.tile_pool(name="v8", bufs=3))
        at8_pool = ctx.enter_context(tc.tile_pool(name="at8", bufs=2))
        const_pool = ctx.enter_context(tc.tile_pool(name="const", bufs=1))
        exp_pool = ctx.enter_context(tc.tile_pool(name="exp", bufs=3))
        at_pool = ctx.enter_context(tc.tile_pool(name="at", bufs=2))
        stat_pool = ctx.enter_context(tc.tile_pool(name="stat", bufs=8))
        o_pool = ctx.enter_context(tc.tile_pool(name="o", bufs=2))
        if use_mask:
            m_pool = ctx.enter_context(tc.tile_pool(name="m", bufs=2))
        psum = ctx.enter_context(tc.tile_pool(name="psum", bufs=4, space="PSUM"))

        # ---- input loads ----
        # Two parallel DMA queues feed the V projection in consumption
        # order. The V loop streams n=0 (first wv half) for a whole round
        # before n=1, so round 0 only needs the wv halves-0 + xt quarter 0.
        # sync:   wv half0 x8, wv half1 x8, m2 x8 (+ mask/out later)
        # gpsimd: xt q0 x8, consts, xt q1, x8 fp8, xt q2, xt q3
        xt = [xt_pool.tile([P, S], BF16, tag="xt", name=f"xt{i}") for i in range(ND)]
        wv = [wv_pool.tile([P, D], BF16, tag="wv", name=f"wv{i}") for i in range(ND)]
        m2 = [m2_pool.tile([P, D], BF16, tag="m2", name=f"m2{i}") for i in range(ND)]
        for d in range(ND):
            for n in range(2):
                nc.sync.dma_start(
                    wv[d][:, n * 512 : (n + 1) * 512],
                    wv_d[d * P : (d + 1) * P, n * 512 : (n + 1) * 512],
                )
        for d in range(ND):
            nc.sync.dma_start(m2[d][:], m2_d[d * P : (d + 1) * P, :])

        def xt_quarter(qt):
            for d in range(ND):
                nc.gpsimd.dma_start(
                    xt[d][:, qt * 512 : (qt + 1) * 512],
                    xt_d[d * P : (d + 1) * P, qt * 512 : (qt + 1) * 512],
                )

        x8 = [
            x8_pool.tile([P, 2 * S], FP8, tag="x8", name=f"x8_{i}")
            for i in range(ND // 2)
        ]
        w2_sb = const_pool.tile([P, ND], F32, tag="w2")
        nc.gpsimd.dma_start(w2_sb[:], w2_d[:, :])
        xt_quarter(0)
        xt_quarter(1)
        for e2 in range(ND // 2):
            for j in range(2):
                nc.gpsimd.dma_start(
                    x8[e2][:, j * S : (j + 1) * S],
                    x8_d[(2 * e2 + j) * P : (2 * e2 + j + 1) * P, :],
                )
        xt_quarter(2)
        xt_quarter(3)

        # ---- V proj: vt[k] = xt^T Wv for all 16 key chunks ----
        # bv is NOT added here: softmax rows sum to 1, so attn@(xWv+bv)/rsum
        # == attn@(xWv)/rsum + bv, and the host adds bv to the final output
        # exactly. Saves 32 rank-1 bias matmuls (~7us of PE).
        # Key chunks 0..9 stay bf16; chunks 10..15 are kept fp8 (paired
        # planes in v8) and consumed by DoubleRow PV matmuls — measured
        # end-to-end rel err 1.68e-2 vs the 2e-2 gate.
        vt = [vt_pool.tile([P, D], BF16, tag="vt", name=f"vt{i}") for i in range(NBF)]
        v8 = [
            v8_pool.tile([P, 2 * D], FP8, tag="v8", name=f"v8_{i}") for i in range(3)
        ]
        for kb in range(4):
            pss = [
                psum.tile([P, D], F32, tag="ps", name=f"vps{kb}_{j}")
                for j in range(4)
            ]
            for d in range(ND):
                for j in range(4):
                    k = kb * 4 + j
                    for n in range(2):
                        nc.tensor.matmul(
                            pss[j][:, n * 512 : (n + 1) * 512],
                            lhsT=xt[d][:, k * P : (k + 1) * P],
                            rhs=wv[d][:, n * 512 : (n + 1) * 512],
                            start=(d == 0),
                            stop=(d == ND - 1),
                        )
            for j in range(4):
                k = kb * 4 + j
                if k < NBF:
                    dst = vt[k][:]
                else:
                    c2, pl = divmod(k - NBF, 2)
                    dst = v8[c2][:, pl * D : (pl + 1) * D]
                if j % 2 == 0:
                    nc.vector.tensor_copy(dst, pss[j][:])
                else:
                    nc.scalar.copy(dst, pss[j][:])

        # ---- TT proj: tt8[e2][:, j*Q:(j+1)*Q] = (M2^T xt_own + w2) fp8 ----
        # One psum per e-chunk, evicted as soon as its group stops, so the
        # evictions spread across the whole phase instead of bursting onto
        # the serial Scalar queue right before the first exp needs it.
        tt8 = [
            tt_pool.tile([P, 2 * Q], FP8, tag="tt", name=f"tt8_{i}")
            for i in range(ND // 2)
        ]
        for e in range(ND):
            pt = psum.tile([P, Q], F32, tag="ps", name=f"tps{e}")
            for d in range(ND):
                for n in range(2):
                    nc.tensor.matmul(
                        pt[:, n * 512 : (n + 1) * 512],
                        lhsT=m2[d][:, e * P : (e + 1) * P],
                        rhs=xt[d][:, qoff + n * 512 : qoff + (n + 1) * 512],
                        start=(d == 0),
                        stop=(d == ND - 1),
                    )
            dst = tt8[e // 2][:, (e % 2) * Q : (e % 2 + 1) * Q]
            if e % 2 == 0:
                nc.scalar.activation(dst, pt[:], AF.Identity, bias=w2_sb[:, e : e + 1])
            else:
                nc.vector.tensor_scalar_add(dst, pt[:], w2_sb[:, e : e + 1])

        tt3 = [t.rearrange("p (j q) -> p j q", j=2) for t in tt8]
        x83 = [t.rearrange("p (j s) -> p j s", j=2) for t in x8]

        # ---- attention, software-pipelined over 8 q-chunks ----
        def scores_phase(qc):
            """fp8 DoubleRow scores + exp(+mask) + row sums for q-chunk qc."""
            exp_sb = exp_pool.tile([P, S], BF16, tag="exp", name=f"exp{qc}")
            sums = stat_pool.tile([P, 2], F32, tag="sums", name=f"sums{qc}")
            for hf in range(2):
                ps = psum.tile([P, Q], F32, tag="ps", name=f"sps{qc}_{hf}")
                for e2 in range(ND // 2):
                    for n in range(2):
                        off = hf * 1024 + n * 512
                        nc.tensor.matmul(
                            ps[:, n * 512 : (n + 1) * 512],
                            lhsT=tt3[e2][:, :, qc * P : (qc + 1) * P],
                            rhs=x83[e2][:, :, off : off + 512],
                            start=(e2 == 0),
                            stop=(e2 == ND // 2 - 1),
                            perf_mode=DR,
                        )
                if use_mask:
                    mt = m_pool.tile([P, Q], F32, tag="m", name=f"mt{qc}_{hf}")
                    nc.sync.dma_start(
                        mt[:], mask_d[qc * P : (qc + 1) * P, hf * 1024 : (hf + 1) * 1024]
                    )
                    nc.vector.tensor_add(ps[:], ps[:], mt[:])
                nc.scalar.activation(
                    exp_sb[:, hf * 1024 : (hf + 1) * 1024],
                    ps[:],
                    AF.Exp,
                    scale=SCALE,
                    accum_out=sums[:, hf : hf + 1],
                )
            return exp_sb, sums

        def pv_phase(qc, exp_sb, sums):
            """transpose + PV + normalized eviction for q-chunk qc."""
            rsum = stat_pool.tile([P, 1], F32, tag="rsum", name=f"rsum{qc}")
            nc.vector.tensor_add(rsum[:], sums[:, 0:1], sums[:, 1:2])
            rinv = stat_pool.tile([P, 1], F32, tag="rinv", name=f"rinv{qc}")
            nc.vector.reciprocal(rinv[:], rsum[:])
            at_sb = at_pool.tile([P, S], BF16, tag="at", name=f"at{qc}")
            # one xbar transpose for all 16 chunks: out[p, c, q] = exp[q, c*128+p].
            # Issued from the sync queue so it never sits behind the exp
            # ACTIVATEs of later chunks on the in-order Scalar queue.
            nc.sync.dma_start(
                out=at_sb.rearrange("p (c q) -> p c q", q=P),
                in_=exp_sb[:, :],
                transpose=True,
            )
            # fp8 copy of the last 4 transposed-attn chunks; the cast runs on
            # the DVE while the PE does the 12 bf16 chunks, so no latency
            at8 = at8_pool.tile([P, (NS - NBF) * P], FP8, tag="at8", name=f"at8_{qc}")
            nc.vector.tensor_copy(at8[:], at_sb[:, NBF * P : S])
            at83 = at8.rearrange("p (c q) -> p c q", q=P)
            pv = psum.tile([P, D], F32, tag="ps", name=f"pv{qc}")
            for k in range(NBF):
                for n in range(2):
                    nc.tensor.matmul(
                        pv[:, n * 512 : (n + 1) * 512],
                        lhsT=at_sb[:, k * P : (k + 1) * P],
                        rhs=vt[k][:, n * 512 : (n + 1) * 512],
                        start=(k == 0),
                        stop=False,
                    )
            for c2 in range(3):
                v83 = v8[c2].rearrange("p (j e) -> p j e", j=2)
                for n in range(2):
                    nc.tensor.matmul(
                        pv[:, n * 512 : (n + 1) * 512],
                        lhsT=at83[:, 2 * c2 : 2 * c2 + 2, :],
                        rhs=v83[:, :, n * 512 : (n + 1) * 512],
                        start=False,
                        stop=(c2 == 2),
                        perf_mode=DR,
                    )
            # split the normalized eviction + store across two engines/queues
            # so the tail after the very last PV matmul is halved
            ot = o_pool.tile([P, D], F32, tag="o", name=f"ot{qc}")
            nc.vector.tensor_scalar_mul(ot[:, 0:512], pv[:, 0:512], rinv[:])
            nc.scalar.mul(ot[:, 512:1024], pv[:, 512:1024], rinv[:])
            nc.sync.dma_start(out_d[qc * P : (qc + 1) * P, 0:512], ot[:, 0:512])
            nc.scalar.dma_start(out_d[qc * P : (qc + 1) * P, 512:1024], ot[:, 512:1024])

        # software pipeline depth 2: scores(qc+1) and scores(qc+2) are
        # emitted before pv(qc), so exp+transpose of chunk qc (≈5.5us of
        # Scalar/xbar latency) complete under two scores phases (7us) and
        # the PE never stalls — including at the first pv.
        from collections import deque

        pend = deque([scores_phase(0), scores_phase(1)])
        for qc in range(NQ):
            if qc + 2 < NQ:
                pend.append(scores_phase(qc + 2))
            pv_phase(qc, *pend.popleft())

    nc.compile()
    return nc


def _get_nc(use_mask: bool) -> bacc.Bacc:
    if use_mask not in _NC_CACHE:
        _NC_CACHE[use_mask] = _build(use_mask)
    return _NC_CACHE[use_mask]


def kernel(x, mask, Wq, bq, Wk, bk, Wv, bv):
    x = np.asarray(x, dtype=np.float32)
    mask = np.asarray(mask, dtype=np.float32)
    Wq = np.asarray(Wq, dtype=np.float32)
    bq = np.asarray(bq, dtype=np.float32)
    Wk = np.asarray(Wk, dtype=np.float32)
    bk = np.asarray(bk, dtype=np.float32)
    Wv = np.asarray(Wv, dtype=np.float32)
    bv = np.asarray(bv, dtype=np.float32)

    B = x.shape[0]
    use_mask = bool(np.any(mask))

    bf = ml_dtypes.bfloat16
    f8 = ml_dtypes.float8_e4m3
    # scores(i,j) = q_i.k_j - alpha_i with M2 = Wq Wk^T, w2 = Wk bq;
    # alpha_i is a per-row constant that softmax drops.
    m2 = (Wq.astype(np.float64) @ Wk.astype(np.float64).T).astype(bf)
    w2 = (Wk.astype(np.float64) @ bq.astype(np.float64)).astype(np.float32)
    w22 = np.ascontiguousarray(w2.reshape(ND, P).T)
    wv_b = Wv.astype(bf)

    nc = _get_nc(use_mask)

    in_maps = []
    for c in range(8):
        b, h = divmod(c, 2)
        # own-half rows first, peer-half rows after (key permutation)
        xr = np.concatenate([x[b, h * Q : (h + 1) * Q], x[b, (1 - h) * Q : (2 - h) * Q]])
        xt = np.ascontiguousarray(xr.T)
        im = {
            "xt": xt.astype(bf),
            "x8": xt.astype(f8),
            "m2": m2,
            "wv": wv_b,
            "w22": w22,
        }
        if use_mask:
            mrows = mask[h * Q : (h + 1) * Q]
            mperm = np.concatenate(
                [mrows[:, h * Q : (h + 1) * Q], mrows[:, (1 - h) * Q : (2 - h) * Q]],
                axis=1,
            )
            im["maskp"] = np.ascontiguousarray(mperm / np.float32(SCALE)).astype(
                np.float32
            )
        in_maps.append(im)

    res = run_bass_kernel_spmd(nc, in_maps, core_ids=list(range(8)))

    out = np.empty((B, S, D), dtype=np.float32)
    for c in range(8):
        b, h = divmod(c, 2)
        out[b, h * Q : (h + 1) * Q, :] = res.results[c]["out"]
    # bv folded out of the kernel (softmax rows sum to 1): add it back here
    out += bv.reshape(1, 1, D)
    return out


# revision 37
# speedup vs baseline: 1.0242x; 1.0242x over previous
"""Distributed single-head attention for Trainium2 (8 NeuronCores).

Problem: B=4, S=2048, D=1024 fp32 attention:
    q = x@Wq+bq; k = x@Wk+bk; v = x@Wv+bv
    out = softmax(q k^T / sqrt(D) + mask) v

Sharding: core c handles batch c//2, query rows [1024*(c%2), +1024).
The host gives every core the FULL x of its batch (both query halves),
so there are NO collectives at all: keys/values for all 2048 positions
are derived locally. The extra cost (projecting the peer half of V,
+128 matmuls) is far cheaper than the serialized in-pair AllGathers +
global CC barrier it replaces (~75us on the old critical path).

Algebraic trick (kept from the earlier version) — the K projection is
eliminated: q_i.k_j = x_i (Wq Wk^T) x_j^T + x_j.(Wk bq) + alpha_i where
alpha_i is a per-row constant softmax drops. With host-precomputed
M2 = Wq@Wk^T and w2 = Wk@bq, scores (up to row const) = TT @ x^T where
TT = x_q @ M2 + w2. One projection instead of two.

Precision: the scores matmul runs in fp8e4m3 with MatmulPerfMode
.DoubleRow (two 128-deep k-planes per instruction -> 2x PE rate).
Measured end-to-end rel err 1.0e-2 (vs 2.1e-3 all-bf16; gate 2e-2).
fp8 anywhere on the V/attn path pushes past the gate, so V proj, TT
proj and PV stay bf16.

Per-core phases (single PE stream, ~800 matmuls):
  V proj:   vt[k][128,1024] = xt^T Wv + bv  for all 16 key chunks
  TT proj:  tt8[e2] fp8 [128, 2*1024] = (M2^T xt_own + w2), d'-pairs
  qc loop (8 query chunks of 128, software-pipelined):
    scores[q,s] fp8 DoubleRow: lhsT=tt8 pairs, rhs=x8 pairs
    e = exp(SCALE*s (+mask)), row-sums via ScalarE accum_out
    attnT via one 3D xbar DMA-transpose (bf16)
    o = attnT^T V (bf16, fp32 PSUM), evicted with *1/rowsum fused
"""

from contextlib import ExitStack

import numpy as np
import ml_dtypes

import concourse.bass as bass
import concourse.tile as tile
import concourse.mybir as mybir
from concourse import bacc
from concourse.bass_utils import run_bass_kernel_spmd

BF16 = mybir.dt.bfloat16
FP8 = mybir.dt.float8e4
F32 = mybir.dt.float32
AF = mybir.ActivationFunctionType
DR = mybir.MatmulPerfMode.DoubleRow

D = 1024  # model dim (= contraction dim)
S = 2048  # full sequence (keys)
Q = 1024  # queries per core
P = 128  # partitions
ND = D // P  # 8 d-chunks
NS = S // P  # 16 key chunks
NQ = Q // P  # 8 query chunks
NBF = 10  # key chunks kept bf16 in PV; the rest go fp8 DoubleRow
SCALE = 1.0 / float(np.sqrt(np.float32(D)))

_NC_CACHE: dict[tuple[bool, int], bacc.Bacc] = {}


def _build(use_mask: bool) -> bacc.Bacc:
    """One program for all cores. The host permutes keys so each core's
    own query rows are ALWAYS xt columns 0..1023 (own keys first, peer
    keys after); softmax+PV are key-permutation invariant and the mask
    columns are permuted to match, so a single program serves both
    query halves."""
    nc = bacc.Bacc("TRN2", target_bir_lowering=False, debug=False, num_devices=8)

    xt_d = nc.dram_tensor("xt", [D, S], BF16, kind="ExternalInput")
    x8_d = nc.dram_tensor("x8", [D, S], FP8, kind="ExternalInput")
    m2_d = nc.dram_tensor("m2", [D, D], BF16, kind="ExternalInput")
    wv_d = nc.dram_tensor("wv", [D, D], BF16, kind="ExternalInput")
    w2_d = nc.dram_tensor("w22", [P, ND], F32, kind="ExternalInput")
    if use_mask:
        mask_d = nc.dram_tensor("maskp", [Q, S], F32, kind="ExternalInput")
    out_d = nc.dram_tensor("out", [Q, D], F32, kind="ExternalOutput")

    qoff = 0  # own query rows are always the first 1024 xt columns

    with tile.TileContext(nc) as tc, ExitStack() as ctx:
        xt_pool = ctx.enter_context(tc.tile_pool(name="xt", bufs=ND))
        x8_pool = ctx.enter_context(tc.tile_pool(name="x8", bufs=ND // 2))
        m2_pool = ctx.enter_context(tc.tile_pool(name="m2", bufs=ND))
        wv_pool = ctx.enter_context(tc.tile_pool(name="wv", bufs=ND))
        tt_pool = ctx.enter_context(tc.tile_pool(name="tt", bufs=ND // 2))
        vt_pool = ctx.enter_context(tc.tile_pool(name="vt", bufs=NBF))
        v8_pool = ctx.enter_context(tc.tile_pool(name="v8", bufs=3))
        at8_pool = ctx.enter_context(tc.tile_pool(name="at8", bufs=2))
        const_pool = ctx.enter_context(tc.tile_pool(name="const", bufs=1))
        exp_pool = ctx.enter_context(tc.tile_pool(name="exp", bufs=3))
        at_pool = ctx.enter_context(tc.tile_pool(name="at", bufs=2))
        stat_pool = ctx.enter_context(tc.tile_pool(name="stat", bufs=8))
        o_pool = ctx.enter_context(tc.tile_pool(name="o", bufs=2))
        if use_mask:
            m_pool = ctx.enter_context(tc.tile_pool(name="m", bufs=2))
        psum = ctx.enter_context(tc.tile_pool(name="psum", bufs=4, space="PSUM"))

        # ---- input loads ----
        # Two parallel DMA queues feed the V projection in consumption
        # order. The V loop streams n=0 (first wv half) for a whole round
        # before n=1, so round 0 only needs the wv halves-0 + xt quarter 0.
        # sync:   wv half0 x8, wv half1 x8, m2 x8 (+ mask/out later)
        # gpsimd: xt q0 x8, consts, xt q1, x8 fp8, xt q2, xt q3
        xt = [xt_pool.tile([P, S], BF16, tag="xt", name=f"xt{i}") for i in range(ND)]
        wv = [wv_pool.tile([P, D], BF16, tag="wv", name=f"wv{i}") for i in range(ND)]
        m2 = [m2_pool.tile([P, D], BF16, tag="m2", name=f"m2{i}") for i in range(ND)]
        for d in range(ND):
            for n in range(2):
                nc.sync.dma_start(
                    wv[d][:, n * 512 : (n + 1) * 512],
                    wv_d[d * P : (d + 1) * P, n * 512 : (n + 1) * 512],
                )
        for d in range(ND):
            nc.sync.dma_start(m2[d][:], m2_d[d * P : (d + 1) * P, :])

        def xt_quarter(qt):
            for d in range(ND):
                nc.gpsimd.dma_start(
                    xt[d][:, qt * 512 : (qt + 1) * 512],
                    xt_d[d * P : (d + 1) * P, qt * 512 : (qt + 1) * 512],
                )

        x8 = [
            x8_pool.tile([P, 2 * S], FP8, tag="x8", name=f"x8_{i}")
            for i in range(ND // 2)
        ]
        w2_sb = const_pool.tile([P, ND], F32, tag="w2")
        nc.gpsimd.dma_start(w2_sb[:], w2_d[:, :])
        xt_quarter(0)
        xt_quarter(1)
        xt_quarter(2)
        xt_quarter(3)
        # x8 is not read until the first scores phase (~90us in): load it
        # last so it never contends with the V-phase xt/wv stream
        for e2 in range(ND // 2):
            for j in range(2):
                nc.gpsimd.dma_start(
                    x8[e2][:, j * S : (j + 1) * S],
                    x8_d[(2 * e2 + j) * P : (2 * e2 + j + 1) * P, :],
                )

        # ---- V proj: vt[k] = xt^T Wv for all 16 key chunks ----
        # bv is NOT added here: softmax rows sum to 1, so attn@(xWv+bv)/rsum
        # == attn@(xWv)/rsum + bv, and the host adds bv to the final output
        # exactly. Saves 32 rank-1 bias matmuls (~7us of PE).
        # Key chunks 0..9 stay bf16; chunks 10..15 are kept fp8 (paired
        # planes in v8) and consumed by DoubleRow PV matmuls — measured
        # end-to-end rel err 1.68e-2 vs the 2e-2 gate.
        vt = [vt_pool.tile([P, D], BF16, tag="vt", name=f"vt{i}") for i in range(NBF)]
        v8 = [
            v8_pool.tile([P, 2 * D], FP8, tag="v8", name=f"v8_{i}") for i in range(3)
        ]
        for kb in range(4):
            pss = [
                psum.tile([P, D], F32, tag="ps", name=f"vps{kb}_{j}")
                for j in range(4)
            ]
            for d in range(ND):
                for j in range(4):
                    k = kb * 4 + j
                    for n in range(2):
                        nc.tensor.matmul(
                            pss[j][:, n * 512 : (n + 1) * 512],
                            lhsT=xt[d][:, k * P : (k + 1) * P],
                            rhs=wv[d][:, n * 512 : (n + 1) * 512],
                            start=(d == 0),
                            stop=(d == ND - 1),
                        )
            for j in range(4):
                k = kb * 4 + j
                if k < NBF:
                    dst = vt[k][:]
                else:
                    c2, pl = divmod(k - NBF, 2)
                    dst = v8[c2][:, pl * D : (pl + 1) * D]
                if j % 2 == 0:
                    nc.vector.tensor_copy(dst, pss[j][:])
                else:
                    nc.scalar.copy(dst, pss[j][:])

        # ---- TT proj: tt8[e2][:, j*Q:(j+1)*Q] = (M2^T xt_own + w2) fp8 ----
        # One psum per e-chunk, evicted as soon as its group stops, so the
        # evictions spread across the whole phase instead of bursting onto
        # the serial Scalar queue right before the first exp needs it.
        tt8 = [
            tt_pool.tile([P, 2 * Q], FP8, tag="tt", name=f"tt8_{i}")
            for i in range(ND // 2)
        ]
        for e in range(ND):
            pt = psum.tile([P, Q], F32, tag="ps", name=f"tps{e}")
            for d in range(ND):
                for n in range(2):
                    nc.tensor.matmul(
                        pt[:, n * 512 : (n + 1) * 512],
                        lhsT=m2[d][:, e * P : (e + 1) * P],
                        rhs=xt[d][:, qoff + n * 512 : qoff + (n + 1) * 512],
                        start=(d == 0),
                        stop=(d == ND - 1),
                    )
            dst = tt8[e // 2][:, (e % 2) * Q : (e % 2 + 1) * Q]
            if e % 2 == 0:
                nc.scalar.activation(dst, pt[:], AF.Identity, bias=w2_sb[:, e : e + 1])
            else:
                nc.vector.tensor_scalar_add(dst, pt[:], w2_sb[:, e : e + 1])

        tt3 = [t.rearrange("p (j q) -> p j q", j=2) for t in tt8]
        x83 = [t.rearrange("p (j s) -> p j s", j=2) for t in x8]

        # ---- attention, software-pipelined over 8 q-chunks ----
        def scores_phase(qc):
            """fp8 DoubleRow scores + exp(+mask) + row sums for q-chunk qc."""
            exp_sb = exp_pool.tile([P, S], BF16, tag="exp", name=f"exp{qc}")
            sums = stat_pool.tile([P, 2], F32, tag="sums", name=f"sums{qc}")
            for hf in range(2):
                ps = psum.tile([P, Q], F32, tag="ps", name=f"sps{qc}_{hf}")
                for e2 in range(ND // 2):
                    for n in range(2):
                        off = hf * 1024 + n * 512
                        nc.tensor.matmul(
                            ps[:, n * 512 : (n + 1) * 512],
                            lhsT=tt3[e2][:, :, qc * P : (qc + 1) * P],
                            rhs=x83[e2][:, :, off : off + 512],
                            start=(e2 == 0),
                            stop=(e2 == ND // 2 - 1),
                            perf_mode=DR,
                        )
                if use_mask:
                    mt = m_pool.tile([P, Q], F32, tag="m", name=f"mt{qc}_{hf}")
                    nc.sync.dma_start(
                        mt[:], mask_d[qc * P : (qc + 1) * P, hf * 1024 : (hf + 1) * 1024]
                    )
                    nc.vector.tensor_add(ps[:], ps[:], mt[:])
                nc.scalar.activation(
                    exp_sb[:, hf * 1024 : (hf + 1) * 1024],
                    ps[:],
                    AF.Exp,
                    scale=SCALE,
                    accum_out=sums[:, hf : hf + 1],
                )
            return exp_sb, sums

        def pv_phase(qc, exp_sb, sums):
            """transpose + PV + normalized eviction for q-chunk qc."""
            rsum = stat_pool.tile([P, 1], F32, tag="rsum", name=f"rsum{qc}")
            nc.vector.tensor_add(rsum[:], sums[:, 0:1], sums[:, 1:2])
            rinv = stat_pool.tile([P, 1], F32, tag="rinv", name=f"rinv{qc}")
            nc.vector.reciprocal(rinv[:], rsum[:])
            at_sb = at_pool.tile([P, S], BF16, tag="at", name=f"at{qc}")
            # one xbar transpose for all 16 chunks: out[p, c, q] = exp[q, c*128+p].
            # Issued from the sync queue so it never sits behind the exp
            # ACTIVATEs of later chunks on the in-order Scalar queue.
            nc.sync.dma_start(
                out=at_sb.rearrange("p (c q) -> p c q", q=P),
                in_=exp_sb[:, :],
                transpose=True,
            )
            # fp8 copy of the last 4 transposed-attn chunks; the cast runs on
            # the DVE while the PE does the 12 bf16 chunks, so no latency
            at8 = at8_pool.tile([P, (NS - NBF) * P], FP8, tag="at8", name=f"at8_{qc}")
            nc.vector.tensor_copy(at8[:], at_sb[:, NBF * P : S])
            at83 = at8.rearrange("p (c q) -> p c q", q=P)
            pv = psum.tile([P, D], F32, tag="ps", name=f"pv{qc}")
            for k in range(NBF):
                for n in range(2):
                    nc.tensor.matmul(
                        pv[:, n * 512 : (n + 1) * 512],
                        lhsT=at_sb[:, k * P : (k + 1) * P],
                        rhs=vt[k][:, n * 512 : (n + 1) * 512],
                        start=(k == 0),
                        stop=False,
                    )
            for c2 in range(3):
                v83 = v8[c2].rearrange("p (j e) -> p j e", j=2)
                for n in range(2):
                    nc.tensor.matmul(
                        pv[:, n * 512 : (n + 1) * 512],
                        lhsT=at83[:, 2 * c2 : 2 * c2 + 2, :],
                        rhs=v83[:, :, n * 512 : (n + 1) * 512],
                        start=False,
                        stop=(c2 == 2),
                        perf_mode=DR,
                    )
            # split the normalized eviction + store across two engines/queues
            # so the tail after the very last PV matmul is halved
            ot = o_pool.tile([P, D], F32, tag="o", name=f"ot{qc}")
            nc.vector.tensor_scalar_mul(ot[:, 0:512], pv[:, 0:512], rinv[:])
            nc.scalar.mul(ot[:, 512:1024], pv[:, 512:1024], rinv[:])
            nc.sync.dma_start(out_d[qc * P : (qc + 1) * P, 0:512], ot[:, 0:512])
            nc.scalar.dma_start(out_d[qc * P : (qc + 1) * P, 512:1024], ot[:, 512:1024])

        # software pipeline depth 2: scores(qc+1) and scores(qc+2) are
        # emitted before pv(qc), so exp+transpose of chunk qc (≈5.5us of
        # Scalar/xbar latency) complete under two scores phases (7us) and
        # the PE never stalls — including at the first pv.
        from collections import deque

        pend = deque([scores_phase(0), scores_phase(1)])
        for qc in range(NQ):
            if qc + 2 < NQ:
                pend.append(scores_phase(qc + 2))
            pv_phase(qc, *pend.popleft())

    nc.compile()
    return nc


def _get_nc(use_mask: bool) -> bacc.Bacc:
    if use_mask not in _NC_CACHE:
        _NC_CACHE[use_mask] = _build(use_mask)
    return _NC_CACHE[use_mask]


def kernel(x, mask, Wq, bq, Wk, bk, Wv, bv):
    x = np.asarray(x, dtype=np.float32)
    mask = np.asarray(mask, dtype=np.float32)
    Wq = np.asarray(Wq, dtype=np.float32)
    bq = np.asarray(bq, dtype=np.float32)
    Wk = np.asarray(Wk, dtype=np.float32)
    bk = np.asarray(bk, dtype=np.float32)
    Wv = np.asarray(Wv, dtype=np.float32)
    bv = np.asarray(bv, dtype=np.float32)

    B = x.shape[0]
    use_mask = bool(np.any(mask))

    bf = ml_dtypes.bfloat16
    f8 = ml_dtypes.float8_e4m3
    # scores(i,j) = q_i.k_j - alpha_i with M2 = Wq Wk^T, w2 = Wk bq;
    # alpha_i is a per-row constant that softmax drops.
    m2 = (Wq.astype(np.float64) @ Wk.astype(np.float64).T).astype(bf)
    w2 = (Wk.astype(np.float64) @ bq.astype(np.float64)).astype(np.float32)
    w22 = np.ascontiguousarray(w2.reshape(ND, P).T)
    wv_b = Wv.astype(bf)

    nc = _get_nc(use_mask)

    in_maps = []
    for c in range(8):
        b, h = divmod(c, 2)
        # own-half rows first, peer-half rows after (key permutation)
        xr = np.concatenate([x[b, h * Q : (h + 1) * Q], x[b, (1 - h) * Q : (2 - h) * Q]])
        xt = np.ascontiguousarray(xr.T)
        im = {
            "xt": xt.astype(bf),
            "x8": xt.astype(f8),
            "m2": m2,
            "wv": wv_b,
            "w22": w22,
        }
        if use_mask:
            mrows = mask[h * Q : (h + 1) * Q]
            mperm = np.concatenate(
                [mrows[:, h * Q : (h + 1) * Q], mrows[:, (1 - h) * Q : (2 - h) * Q]],
                axis=1,
            )
            im["maskp"] = np.ascontiguousarray(mperm / np.float32(SCALE)).astype(
                np.float32
            )
        in_maps.append(im)

    res = run_bass_kernel_spmd(nc, in_maps, core_ids=list(range(8)))

    out = np.empty((B, S, D), dtype=np.float32)
    for c in range(8):
        b, h = divmod(c, 2)
        out[b, h * Q : (h + 1) * Q, :] = res.results[c]["out"]
    # bv folded out of the kernel (softmax rows sum to 1): add it back here
    out += bv.reshape(1, 1, D)
    return out
